# revision 2
# baseline (speedup 1.0000x reference)
"""Binarized 3x3 conv via mixed-precision taps on 8 TRN2 cores.

Math: out = conv2d(clip(x,-1,1), concat(clip(reweight,-1,1), conv_w)), pad=1
= one 128->128 3x3 conv.  Data-parallel: 4 images/core, weights replicated.

Per-core kernel: a subset of the 9 taps is computed with e4m3 fp8 operands in
DoubleRow perf mode (2 taps per matmul, ~2x PE throughput); the remaining taps
run in fp16 (exact).  Host pre-clips and pre-quantizes activations; fp8
activations are DMA'd into NP spatially-shifted copies ("planes") of a padded
SBUF buffer so a DoubleRow pair (2 taps at different offsets) reads two
adjacent planes with one natural strided AP [C, 2, R, W].

Per-channel scales: weights are scaled by s_co/s_ci before quantization and
activations by s_ci; the PSUM->SBUF copy multiplies by 1/s_co per partition.
fp16-tap weights carry s_co so all taps accumulate at the same scale.
"""

import time as _time

import numpy as np
import ml_dtypes
from contextlib import ExitStack

import concourse.bass as bass
import concourse.mybir as mybir
import concourse.tile as tile
from concourse import bacc

B, C, H, W = 32, 128, 112, 112
NCORES = 8
BPC = B // NCORES
R = 4  # output rows per PSUM block

GUARD = 2          # extra zero margin (rows/cols) so shifted planes stay in-bounds
HP = H + 2 + 2 * GUARD   # 118 buffer rows
WP = 128                 # padded row stride (bytes for fp8); cols 0..117 used
ROW0 = GUARD + 1         # buffer row of image row 0 (plane shift 0)
COL0 = GUARD + 1         # buffer col of image col 0

# ---- tap config (overridden by the optimizer via set_config) ----
# pairs: list of ((khA,kwA),(khB,kwB), plane_lo); requires
#   SIGMA[plane_lo+1]-SIGMA[plane_lo] == tapA - tapB
SIGMA = [(0, 0), (0, -2), (-2, -2)]
PAIRS = [((0, -1), (0, 1), 0), ((-1, 0), (1, 0), 1)]
TAPS16 = [(-1, -1), (-1, 1), (0, 0), (1, -1), (1, 1)]
S_CO = np.ones(C, np.float32)
S_CI = np.ones(C, np.float32)
OUT_DT = "fp16"  # "fp16" | "f32"

LOOP_MODE = "group"  # "block" | "group" (tap-outer over 8 PSUM banks)
GROUP = 8
PAIR_PERF = "drsw"  # "dr" | "drsw" (DoubleRowSwInterleave: contiguous weight loads)

MODE = "dr"

_nc_cache: dict = {}
_runner_cache: dict = {}


def set_config(sigma, pairs, taps16, s_co, s_ci, out_dt="fp16"):
    global SIGMA, PAIRS, TAPS16, S_CO, S_CI, OUT_DT, _nc_cache, _runner_cache
    SIGMA, PAIRS, TAPS16 = sigma, pairs, taps16
    S_CO = np.asarray(s_co, np.float32)
    S_CI = np.asarray(s_ci, np.float32)
    OUT_DT = out_dt
    _nc_cache, _runner_cache = {}, {}


def _build(mode: str = "dr", bpc: int = BPC, h: int = H, w: int = W, reps: int = 1) -> bass.Bass:
    f32 = mybir.dt.float32
    fp16 = mybir.dt.float16
    fp8 = mybir.dt.float8e4
    out_dt = fp16 if OUT_DT == "fp16" else f32
    NP = len(SIGMA)
    NPAIR = len(PAIRS)
    N16 = len(TAPS16)
    assert h % R == 0

    w8_shape = (
        [C, max(NPAIR, 1), C, 2] if PAIR_PERF == "drsw" else [C, max(NPAIR, 1), 2, C]
    )
    nc = bacc.Bacc("TRN2", target_bir_lowering=False, debug=False)
    x8_in = nc.declare_dram_parameter("x8", [bpc, C, h, w], fp8, isOutput=False)
    x16_in = nc.declare_dram_parameter("x16", [bpc, C, h, w], fp16, isOutput=False)
    w8_in = nc.declare_dram_parameter("w8", w8_shape, fp8, isOutput=False)
    w16_in = nc.declare_dram_parameter("w16", [C, max(N16, 1), C], fp16, isOutput=False)
    sinv_in = nc.declare_dram_parameter("sinv", [C, 1], f32, isOutput=False)
    out_d = nc.declare_dram_parameter("out", [bpc, C, h, w], out_dt, isOutput=True)

    pair_perf = (
        mybir.MatmulPerfMode.DoubleRowSwInterleave
        if PAIR_PERF == "drsw"
        else mybir.MatmulPerfMode.DoubleRow
    )
    with tile.TileContext(nc) as tc, ExitStack() as ctx:
        wpool = ctx.enter_context(tc.tile_pool(name="wpool", bufs=1))
        apool = ctx.enter_context(tc.tile_pool(name="apool", bufs=1))
        opool = ctx.enter_context(tc.tile_pool(name="opool", bufs=10))
        pspool = ctx.enter_context(tc.tile_pool(name="pspool", bufs=8, space="PSUM"))

        w8_s = wpool.tile(w8_shape, fp8)
        w16_s = wpool.tile([C, max(N16, 1), C], fp16)
        sinv_s = wpool.tile([C, 1], f32)
        nc.sync.dma_start(w8_s[:], w8_in[:])
        nc.sync.dma_start(w16_s[:], w16_in[:])
        nc.sync.dma_start(sinv_s[:], sinv_in[:])

        # PE warmup against the HAM clock gate during the input-DMA lead-in
        warm = wpool.tile([C, R * w], fp16)
        nc.vector.memset(warm[:], 0.0)
        wps = pspool.tile([C, R * w], f32, tag="ps")
        for _wi in range(6):
            nc.tensor.matmul(wps[:], warm[:, :C], warm[:], start=True, stop=True)

        for b_outer in range(bpc * reps):
            b = b_outer % bpc
            a8 = apool.tile([C, NP, HP, WP], fp8, tag="a8", bufs=2)
            a16 = apool.tile([C, HP, WP], fp16, tag="a16", bufs=2)

            # zero the guard borders of each plane (only rows/cols a tap can read)
            for p in range(NP):
                sh, sw = SIGMA[p]
                r_img0, r_img1 = ROW0 + sh, ROW0 + sh + h  # image row span in buffer
                c_img0, c_img1 = COL0 + sw, COL0 + sw + w
                nc.vector.memset(a8[:, p, 0:r_img0, :118], 0.0)
                nc.vector.memset(a8[:, p, r_img1:118, :118], 0.0)
                nc.vector.memset(a8[:, p, r_img0:r_img1, 0:c_img0], 0.0)
                nc.vector.memset(a8[:, p, r_img0:r_img1, c_img1:118], 0.0)
            nc.vector.memset(a16[:, 0:ROW0, :118], 0.0)
            nc.vector.memset(a16[:, ROW0 + h : 118, :118], 0.0)
            nc.vector.memset(a16[:, ROW0 : ROW0 + h, 0:COL0], 0.0)
            nc.vector.memset(a16[:, ROW0 : ROW0 + h, COL0 + w : 118], 0.0)

            sched = [16] * (h // 16)
            r0 = 0
            for sz in sched:
                r1 = r0 + sz
                for p in range(NP):
                    sh, sw = SIGMA[p]
                    nc.sync.dma_start(
                        a8[:, p, ROW0 + sh + r0 : ROW0 + sh + r1, COL0 + sw : COL0 + sw + w],
                        x8_in[b][:, r0:r1, :],
                    )
                nc.sync.dma_start(
                    a16[:, ROW0 + r0 : ROW0 + r1, COL0 : COL0 + w],
                    x16_in[b][:, r0:r1, :],
                )
                r0 = r1

            out_flat = out_d[b].rearrange("c h w -> c (h w)")
            n_mm = NPAIR + N16

            def dr_rhs(j, h0):
                tA, tB, pl = PAIRS[j]
                khA, kwA = tA
                sh, sw = SIGMA[pl]
                rb = ROW0 + h0 + khA + sh
                cb = COL0 + kwA + sw
                return a8[:, pl : pl + 2, rb : rb + R, cb : cb + w]

            def f16_rhs(i, h0):
                kh, kw = TAPS16[i]
                return a16[:, ROW0 + h0 + kh : ROW0 + h0 + kh + R,
                           COL0 + kw : COL0 + kw + w]

            def emit_copy(ps, h0):
                ot = opool.tile([C, R * w], out_dt)
                nc.scalar.activation(
                    ot[:], ps[:], mybir.ActivationFunctionType.Copy,
                    scale=sinv_s[:],
                )
                nc.scalar.dma_start(out_flat[:, h0 * w : (h0 + R) * w], ot[:])

            if LOOP_MODE == "block":
                for h0 in range(0, h, R):
                    ps = pspool.tile([C, R * w], f32)
                    i_mm = 0
                    for j in range(NPAIR):
                        nc.tensor.matmul(
                            ps[:], w8_s[:, j], dr_rhs(j, h0),
                            start=(i_mm == 0), stop=(i_mm == n_mm - 1),
                            perf_mode=pair_perf,
                        )
                        i_mm += 1
                    for i in range(N16):
                        nc.tensor.matmul(
                            ps[:], w16_s[:, i], f16_rhs(i, h0),
                            start=(i_mm == 0), stop=(i_mm == n_mm - 1),
                        )
                        i_mm += 1
                    emit_copy(ps, h0)
            else:  # "group": tap-outer over GROUP psum banks, 1 weight load per tap
                for g0 in range(0, h, R * GROUP):
                    h0s = list(range(g0, min(g0 + R * GROUP, h), R))
                    pss = [
                        pspool.tile([C, R * w], f32, name="ps", tag="ps")
                        for _ in h0s
                    ]
                    i_mm = 0
                    for j in range(NPAIR):
                        for bi, h0 in enumerate(h0s):
                            nc.tensor.matmul(
                                pss[bi][:], w8_s[:, j], dr_rhs(j, h0),
                                start=(i_mm == 0), stop=(i_mm == n_mm - 1),
                                perf_mode=pair_perf,
                                skip_group_check=True,
                            )
                        i_mm += 1
                    for i in range(N16):
                        for bi, h0 in enumerate(h0s):
                            nc.tensor.matmul(
                                pss[bi][:], w16_s[:, i], f16_rhs(i, h0),
                                start=(i_mm == 0), stop=(i_mm == n_mm - 1),
                                skip_group_check=True,
                            )
                        i_mm += 1
                    for bi, h0 in enumerate(h0s):
                        emit_copy(pss[bi], h0)

    nc.compile()
    return nc


def _prep_weights(reweight: np.ndarray, conv_w: np.ndarray):
    """Returns (w8, w16, sinv) device tensors from the full-precision weights."""
    w_full = np.concatenate(
        [np.clip(reweight, -1.0, 1.0), conv_w], axis=0
    ).astype(np.float32)  # [co, ci, 3, 3]
    NPAIR, N16 = len(PAIRS), len(TAPS16)
    # effective per-weight scale: s_co / s_ci  (activation carries s_ci)
    g = S_CO[None, :] / S_CI[:, None]  # [ci, co]
    if PAIR_PERF == "drsw":
        # interleaved layout: [ci, pair, p, j]: elem (p, 0) = A[:, 127-p],
        # (p, 1) = B[:, 127-p]  (A/B pairs adjacent, columns reversed)
        w8 = np.zeros((C, max(NPAIR, 1), C, 2), ml_dtypes.float8_e4m3)
        for j, (tA, tB, _pl) in enumerate(PAIRS):
            for k, (kh, kw) in enumerate((tA, tB)):
                q = (w_full[:, :, kh + 1, kw + 1].T * g).astype(
                    ml_dtypes.float8_e4m3
                )  # [ci, co]
                w8[:, j, :, k] = q[:, ::-1]
    else:
        w8 = np.zeros((C, max(NPAIR, 1), 2, C), ml_dtypes.float8_e4m3)
        for j, (tA, tB, _pl) in enumerate(PAIRS):
            for k, (kh, kw) in enumerate((tA, tB)):
                # lhsT layout [ci, pair, k, co]
                w8[:, j, k, :] = (w_full[:, :, kh + 1, kw + 1].T * g).astype(
                    ml_dtypes.float8_e4m3
                )
    w16 = np.zeros((C, max(N16, 1), C), np.float16)
    for i, (kh, kw) in enumerate(TAPS16):
        w16[:, i, :] = (w_full[:, :, kh + 1, kw + 1].T * S_CO[None, :]).astype(
            np.float16
        )
    sinv = (1.0 / S_CO).astype(np.float32).reshape(C, 1)
    return w8, w16, sinv


def _prep_acts(x: np.ndarray):
    a = np.clip(np.asarray(x, np.float32), -1.0, 1.0)
    a8 = (a * S_CI[None, :, None, None]).astype(ml_dtypes.float8_e4m3)
    a16 = a.astype(np.float16)
    return a8, a16


def make_in_maps(x, reweight, conv_w):
    w8, w16, sinv = _prep_weights(np.asarray(reweight), np.asarray(conv_w))
    a8, a16 = _prep_acts(x)
    return [
        {
            "x8": np.ascontiguousarray(a8[i * BPC : (i + 1) * BPC]),
            "x16": np.ascontiguousarray(a16[i * BPC : (i + 1) * BPC]),
            "w8": w8, "w16": w16, "sinv": sinv,
        }
        for i in range(NCORES)
    ]


class _Runner:
    """Persistent jitted shard_map executor (same as baseline kernel.py)."""

    def __init__(self, nc, n_cores: int):
        import jax
        from concourse import bass2jax
        from jax.experimental.shard_map import shard_map
        from jax.sharding import Mesh, NamedSharding, PartitionSpec

        bass2jax.install_neuronx_cc_hook()
        self.jax = jax
        self.n_cores = n_cores
        partition_name = nc.partition_id_tensor.name if nc.partition_id_tensor else None
        in_names, out_names, out_avals = [], [], []
        for alloc in nc.m.functions[0].allocations:
            if not isinstance(alloc, mybir.MemoryLocationSet):
                continue
            name = alloc.memorylocations[0].name
            if alloc.kind == "ExternalInput":
                if name != partition_name:
                    in_names.append(name)
            elif alloc.kind == "ExternalOutput":
                out_names.append(name)
                out_avals.append(
                    jax.core.ShapedArray(
                        tuple(alloc.tensor_shape), mybir.dt.np(alloc.dtype)
                    )
                )
        self.in_names, self.out_names, self.out_avals = in_names, out_names, out_avals
        n_params = len(in_names)
        all_in_names = list(in_names) + list(out_names)
        if partition_name is not None:
            all_in_names.append(partition_name)
        donate = tuple(range(n_params, n_params + len(out_names)))

        def _body(*args):
            operands = list(args)
            if partition_name is not None:
                operands.append(bass2jax.partition_id_tensor())
            return tuple(
                bass2jax._bass_exec_p.bind(
                    *operands,
                    out_avals=tuple(out_avals),
                    in_names=tuple(all_in_names),
                    out_names=tuple(out_names),
                    lowering_input_output_aliases=(),
                    sim_require_finite=True,
                    sim_require_nnan=True,
                    nc=nc,
                )
            )

        devices = jax.devices()[:n_cores]
        assert len(devices) >= n_cores, f"need {n_cores} devices, got {len(devices)}"
        mesh = Mesh(np.asarray(devices), ("core",))
        self.sharding = NamedSharding(mesh, PartitionSpec("core"))
        self.sharded = jax.jit(
            shard_map(
                _body, mesh=mesh,
                in_specs=(PartitionSpec("core"),) * (n_params + len(out_names)),
                out_specs=(PartitionSpec("core"),) * len(out_names),
                check_rep=False,
            ),
            donate_argnums=donate, keep_unused=True,
        )
        self._outs = None

    def __call__(self, in_maps):
        jax = self.jax
        per_core = [[np.asarray(m[name]) for name in self.in_names] for m in in_maps]
        concat_in = [
            np.concatenate([per_core[c][i] for c in range(self.n_cores)], axis=0)
            for i in range(len(self.in_names))
        ]
        xin = [jax.device_put(a, self.sharding) for a in concat_in]
        if self._outs is None:
            self._outs = [
                jax.device_put(
                    np.zeros((self.n_cores * av.shape[0], *av.shape[1:]), av.dtype),
                    self.sharding,
                )
                for av in self.out_avals
            ]
        self._outs = list(self.sharded(*xin, *self._outs))
        out_np = [np.asarray(o) for o in self._outs]
        return [
            {
                name: out_np[i].reshape(self.n_cores, *self.out_avals[i].shape)[c]
                for i, name in enumerate(self.out_names)
            }
            for c in range(self.n_cores)
        ]


def _get_nc(mode: str = "dr"):
    key = (mode, LOOP_MODE)
    if key not in _nc_cache:
        _nc_cache[key] = _build(mode)
    return _nc_cache[key]


def _run_spmd(nc, in_maps, mode: str = "dr"):
    last = None
    key = (mode, LOOP_MODE)
    for attempt in range(3):
        try:
            if key not in _runner_cache:
                _runner_cache[key] = _Runner(nc, NCORES)
            return _runner_cache[key](in_maps)
        except Exception as e:
            last = e
            _runner_cache.pop(key, None)
        try:
            from concourse.bass_utils import run_bass_kernel_spmd

            return run_bass_kernel_spmd(nc, in_maps, list(range(NCORES))).results
        except Exception as e:
            last = e
            _time.sleep(15)
    raise last


def kernel(x, reweight, conv_w):
    nc = _get_nc()
    in_maps = make_in_maps(x, reweight, conv_w)
    results = _run_spmd(nc, in_maps)
    out = np.concatenate([results[i]["out"] for i in range(NCORES)], axis=0)
    return np.ascontiguousarray(out.astype(np.float32))


# revision 4
# speedup vs baseline: 1.1278x; 1.1278x over previous
"""Binarized 3x3 conv via mixed-precision taps on 8 TRN2 cores.

Math: out = conv2d(clip(x,-1,1), concat(clip(reweight,-1,1), conv_w)), pad=1
= one 128->128 3x3 conv.  Data-parallel: 4 images/core, weights replicated.

Per-core kernel: 4 of the 9 taps are computed with e4m3 fp8 operands in
DoubleRowSwInterleave perf mode (2 taps per matmul with host-pre-interleaved
weights for contiguous weight loads); the remaining 5 taps run in fp16
(exact).  Verified absmax-rel error vs the f32 reference: 1.850e-2 (< 2e-2),
bit-identical to the host-side numpy simulation of the same quantization.

Host pre-clips and pre-quantizes activations; fp8 activations are DMA'd into
3 spatially-shifted copies ("planes") of a padded SBUF buffer so a DoubleRow
pair (2 taps at different offsets) reads two adjacent planes with one natural
strided AP [C, 2, R, W].  Input ships as fp8+fp16 (19.2 MB/core vs 25.7 f32)
and output as fp16, roughly halving HBM traffic vs the f32 baseline.

Matmuls are issued tap-outer over groups of 8 PSUM banks ("group" loop) so
each weight load serves 8 matmuls — weight loads are NOT hidden behind
matmuls on this setup (measured +53ns/matmul fp16, +213ns per DoubleRow
load), so amortizing and shrinking them is what the loop structure and the
SwInterleave layout buy.  Measured (chained-dispatch slope): 204-216 us/rep
vs 232 us for the naive block loop and ~250-265 us for the staged fp16
baseline on the same hardware session.

Per-channel scales: weights are scaled by s_co/s_ci before quantization and
activations by s_ci; the PSUM->SBUF copy multiplies by 1/s_co per partition.
fp16-tap weights carry s_co so all taps accumulate at the same scale.
"""

import time as _time

import numpy as np
import ml_dtypes
from contextlib import ExitStack

import concourse.bass as bass
import concourse.mybir as mybir
import concourse.tile as tile
from concourse import bacc

B, C, H, W = 32, 128, 112, 112
NCORES = 8
BPC = B // NCORES
R = 4  # output rows per PSUM block

GUARD = 2          # extra zero margin (rows/cols) so shifted planes stay in-bounds
HP = H + 2 + 2 * GUARD   # 118 buffer rows
WP = 128                 # padded row stride (bytes for fp8); cols 0..117 used
ROW0 = GUARD + 1         # buffer row of image row 0 (plane shift 0)
COL0 = GUARD + 1         # buffer col of image col 0

# ---- tap config (overridden by the optimizer via set_config) ----
# pairs: list of ((khA,kwA),(khB,kwB), plane_lo); requires
#   SIGMA[plane_lo+1]-SIGMA[plane_lo] == tapA - tapB
SIGMA = [(0, 0), (-2, 0)]
PAIRS = [((-1, -1), (1, -1), 0), ((-1, 1), (1, 1), 0)]
TAPS16 = [(-1, 0), (0, -1), (0, 0), (0, 1), (1, 0)]
S_CO = np.ones(C, np.float32)
S_CI = np.ones(C, np.float32)
OUT_DT = "fp16"  # "fp16" | "f32"

LOOP_MODE = "group"  # "block" | "group" (tap-outer over 8 PSUM banks)
GROUP = 8
PAIR_PERF = "drsw"  # "dr" | "drsw" (DoubleRowSwInterleave: contiguous weight loads)

MODE = "dr"

_nc_cache: dict = {}
_runner_cache: dict = {}


def set_config(sigma, pairs, taps16, s_co, s_ci, out_dt="fp16"):
    global SIGMA, PAIRS, TAPS16, S_CO, S_CI, OUT_DT, _nc_cache, _runner_cache
    SIGMA, PAIRS, TAPS16 = sigma, pairs, taps16
    S_CO = np.asarray(s_co, np.float32)
    S_CI = np.asarray(s_ci, np.float32)
    OUT_DT = out_dt
    _nc_cache, _runner_cache = {}, {}


def _build(mode: str = "dr", bpc: int = BPC, h: int = H, w: int = W, reps: int = 1) -> bass.Bass:
    f32 = mybir.dt.float32
    fp16 = mybir.dt.float16
    fp8 = mybir.dt.float8e4
    out_dt = fp16 if OUT_DT == "fp16" else f32
    NP = len(SIGMA)
    NPAIR = len(PAIRS)
    N16 = len(TAPS16)
    assert h % R == 0

    w8_shape = (
        [C, max(NPAIR, 1), C, 2] if PAIR_PERF == "drsw" else [C, max(NPAIR, 1), 2, C]
    )
    nc = bacc.Bacc("TRN2", target_bir_lowering=False, debug=False)
    x8_in = nc.declare_dram_parameter("x8", [bpc, C, h, w], fp8, isOutput=False)
    x16_in = nc.declare_dram_parameter("x16", [bpc, C, h, w], fp16, isOutput=False)
    w8_in = nc.declare_dram_parameter("w8", w8_shape, fp8, isOutput=False)
    w16_in = nc.declare_dram_parameter("w16", [C, max(N16, 1), C], fp16, isOutput=False)
    sinv_in = nc.declare_dram_parameter("sinv", [C, 1], f32, isOutput=False)
    out_d = nc.declare_dram_parameter("out", [bpc, C, h, w], out_dt, isOutput=True)

    pair_perf = (
        mybir.MatmulPerfMode.DoubleRowSwInterleave
        if PAIR_PERF == "drsw"
        else mybir.MatmulPerfMode.DoubleRow
    )
    with tile.TileContext(nc) as tc, ExitStack() as ctx:
        wpool = ctx.enter_context(tc.tile_pool(name="wpool", bufs=1))
        apool = ctx.enter_context(tc.tile_pool(name="apool", bufs=1))
        opool = ctx.enter_context(tc.tile_pool(name="opool", bufs=10))
        pspool = ctx.enter_context(tc.tile_pool(name="pspool", bufs=8, space="PSUM"))

        w8_s = wpool.tile(w8_shape, fp8)
        w16_s = wpool.tile([C, max(N16, 1), C], fp16)
        sinv_s = wpool.tile([C, 1], f32)
        nc.sync.dma_start(w8_s[:], w8_in[:])
        nc.sync.dma_start(w16_s[:], w16_in[:])
        nc.sync.dma_start(sinv_s[:], sinv_in[:])

        # PE warmup against the HAM clock gate during the input-DMA lead-in
        warm = wpool.tile([C, R * w], fp16)
        nc.vector.memset(warm[:], 0.0)
        wps = pspool.tile([C, R * w], f32, tag="ps")
        for _wi in range(6):
            nc.tensor.matmul(wps[:], warm[:, :C], warm[:], start=True, stop=True)

        for b_outer in range(bpc * reps):
            b = b_outer % bpc
            a8 = apool.tile([C, NP, HP, WP], fp8, tag="a8", bufs=2)
            a16 = apool.tile([C, HP, WP], fp16, tag="a16", bufs=2)

            # zero the guard borders of each plane (only rows/cols a tap can read)
            for p in range(NP):
                sh, sw = SIGMA[p]
                r_img0, r_img1 = ROW0 + sh, ROW0 + sh + h  # image row span in buffer
                c_img0, c_img1 = COL0 + sw, COL0 + sw + w
                nc.vector.memset(a8[:, p, 0:r_img0, :118], 0.0)
                nc.vector.memset(a8[:, p, r_img1:118, :118], 0.0)
                nc.vector.memset(a8[:, p, r_img0:r_img1, 0:c_img0], 0.0)
                nc.vector.memset(a8[:, p, r_img0:r_img1, c_img1:118], 0.0)
            nc.vector.memset(a16[:, 0:ROW0, :118], 0.0)
            nc.vector.memset(a16[:, ROW0 + h : 118, :118], 0.0)
            nc.vector.memset(a16[:, ROW0 : ROW0 + h, 0:COL0], 0.0)
            nc.vector.memset(a16[:, ROW0 : ROW0 + h, COL0 + w : 118], 0.0)

            sched = [h]  # one chunk per tensor: fewest DMA descriptors/sems,
            # and maximal slack between a chunk's semaphore and first PE read
            r0 = 0
            for sz in sched:
                r1 = r0 + sz
                for p in range(NP):
                    sh, sw = SIGMA[p]
                    nc.sync.dma_start(
                        a8[:, p, ROW0 + sh + r0 : ROW0 + sh + r1, COL0 + sw : COL0 + sw + w],
                        x8_in[b][:, r0:r1, :],
                    )
                nc.sync.dma_start(
                    a16[:, ROW0 + r0 : ROW0 + r1, COL0 : COL0 + w],
                    x16_in[b][:, r0:r1, :],
                )
                r0 = r1

            out_flat = out_d[b].rearrange("c h w -> c (h w)")
            n_mm = NPAIR + N16

            def dr_rhs(j, h0):
                tA, tB, pl = PAIRS[j]
                khA, kwA = tA
                sh, sw = SIGMA[pl]
                rb = ROW0 + h0 + khA + sh
                cb = COL0 + kwA + sw
                return a8[:, pl : pl + 2, rb : rb + R, cb : cb + w]

            def f16_rhs(i, h0):
                kh, kw = TAPS16[i]
                return a16[:, ROW0 + h0 + kh : ROW0 + h0 + kh + R,
                           COL0 + kw : COL0 + kw + w]

            def emit_copy(ps, h0):
                ot = opool.tile([C, R * w], out_dt)
                nc.scalar.activation(
                    ot[:], ps[:], mybir.ActivationFunctionType.Copy,
                    scale=sinv_s[:],
                )
                nc.scalar.dma_start(out_flat[:, h0 * w : (h0 + R) * w], ot[:])

            if LOOP_MODE == "block":
                for h0 in range(0, h, R):
                    ps = pspool.tile([C, R * w], f32)
                    i_mm = 0
                    for j in range(NPAIR):
                        nc.tensor.matmul(
                            ps[:], w8_s[:, j], dr_rhs(j, h0),
                            start=(i_mm == 0), stop=(i_mm == n_mm - 1),
                            perf_mode=pair_perf,
                        )
                        i_mm += 1
                    for i in range(N16):
                        nc.tensor.matmul(
                            ps[:], w16_s[:, i], f16_rhs(i, h0),
                            start=(i_mm == 0), stop=(i_mm == n_mm - 1),
                        )
                        i_mm += 1
                    emit_copy(ps, h0)
            else:  # "group": tap-outer over GROUP psum banks, 1 weight load per tap
                for g0 in range(0, h, R * GROUP):
                    h0s = list(range(g0, min(g0 + R * GROUP, h), R))
                    pss = [
                        pspool.tile([C, R * w], f32, name="ps", tag="ps")
                        for _ in h0s
                    ]
                    i_mm = 0
                    for j in range(NPAIR):
                        for bi, h0 in enumerate(h0s):
                            nc.tensor.matmul(
                                pss[bi][:], w8_s[:, j], dr_rhs(j, h0),
                                start=(i_mm == 0), stop=(i_mm == n_mm - 1),
                                perf_mode=pair_perf,
                                skip_group_check=True,
                            )
                        i_mm += 1
                    for i in range(N16):
                        for bi, h0 in enumerate(h0s):
                            nc.tensor.matmul(
                                pss[bi][:], w16_s[:, i], f16_rhs(i, h0),
                                start=(i_mm == 0), stop=(i_mm == n_mm - 1),
                                skip_group_check=True,
                            )
                        i_mm += 1
                    for bi, h0 in enumerate(h0s):
                        emit_copy(pss[bi], h0)

    nc.compile()
    return nc


def _prep_weights(reweight: np.ndarray, conv_w: np.ndarray):
    """Returns (w8, w16, sinv) device tensors from the full-precision weights."""
    w_full = np.concatenate(
        [np.clip(reweight, -1.0, 1.0), conv_w], axis=0
    ).astype(np.float32)  # [co, ci, 3, 3]
    NPAIR, N16 = len(PAIRS), len(TAPS16)
    # effective per-weight scale: s_co / s_ci  (activation carries s_ci)
    g = S_CO[None, :] / S_CI[:, None]  # [ci, co]
    if PAIR_PERF == "drsw":
        # interleaved layout: [ci, pair, p, j]: elem (p, 0) = A[:, 127-p],
        # (p, 1) = B[:, 127-p]  (A/B pairs adjacent, columns reversed)
        w8 = np.zeros((C, max(NPAIR, 1), C, 2), ml_dtypes.float8_e4m3)
        for j, (tA, tB, _pl) in enumerate(PAIRS):
            for k, (kh, kw) in enumerate((tA, tB)):
                q = (w_full[:, :, kh + 1, kw + 1].T * g).astype(
                    ml_dtypes.float8_e4m3
                )  # [ci, co]
                w8[:, j, :, k] = q[:, ::-1]
    else:
        w8 = np.zeros((C, max(NPAIR, 1), 2, C), ml_dtypes.float8_e4m3)
        for j, (tA, tB, _pl) in enumerate(PAIRS):
            for k, (kh, kw) in enumerate((tA, tB)):
                # lhsT layout [ci, pair, k, co]
                w8[:, j, k, :] = (w_full[:, :, kh + 1, kw + 1].T * g).astype(
                    ml_dtypes.float8_e4m3
                )
    w16 = np.zeros((C, max(N16, 1), C), np.float16)
    for i, (kh, kw) in enumerate(TAPS16):
        w16[:, i, :] = (w_full[:, :, kh + 1, kw + 1].T * S_CO[None, :]).astype(
            np.float16
        )
    sinv = (1.0 / S_CO).astype(np.float32).reshape(C, 1)
    return w8, w16, sinv


def _prep_acts(x: np.ndarray):
    a = np.clip(np.asarray(x, np.float32), -1.0, 1.0)
    a8 = (a * S_CI[None, :, None, None]).astype(ml_dtypes.float8_e4m3)
    a16 = a.astype(np.float16)
    return a8, a16


def make_in_maps(x, reweight, conv_w):
    w8, w16, sinv = _prep_weights(np.asarray(reweight), np.asarray(conv_w))
    a8, a16 = _prep_acts(x)
    return [
        {
            "x8": np.ascontiguousarray(a8[i * BPC : (i + 1) * BPC]),
            "x16": np.ascontiguousarray(a16[i * BPC : (i + 1) * BPC]),
            "w8": w8, "w16": w16, "sinv": sinv,
        }
        for i in range(NCORES)
    ]


class _Runner:
    """Persistent jitted shard_map executor (same as baseline kernel.py)."""

    def __init__(self, nc, n_cores: int):
        import jax
        from concourse import bass2jax
        from jax.experimental.shard_map import shard_map
        from jax.sharding import Mesh, NamedSharding, PartitionSpec

        bass2jax.install_neuronx_cc_hook()
        self.jax = jax
        self.n_cores = n_cores
        partition_name = nc.partition_id_tensor.name if nc.partition_id_tensor else None
        in_names, out_names, out_avals = [], [], []
        for alloc in nc.m.functions[0].allocations:
            if not isinstance(alloc, mybir.MemoryLocationSet):
                continue
            name = alloc.memorylocations[0].name
            if alloc.kind == "ExternalInput":
                if name != partition_name:
                    in_names.append(name)
            elif alloc.kind == "ExternalOutput":
                out_names.append(name)
                out_avals.append(
                    jax.core.ShapedArray(
                        tuple(alloc.tensor_shape), mybir.dt.np(alloc.dtype)
                    )
                )
        self.in_names, self.out_names, self.out_avals = in_names, out_names, out_avals
        n_params = len(in_names)
        all_in_names = list(in_names) + list(out_names)
        if partition_name is not None:
            all_in_names.append(partition_name)
        donate = tuple(range(n_params, n_params + len(out_names)))

        def _body(*args):
            operands = list(args)
            if partition_name is not None:
                operands.append(bass2jax.partition_id_tensor())
            return tuple(
                bass2jax._bass_exec_p.bind(
                    *operands,
                    out_avals=tuple(out_avals),
                    in_names=tuple(all_in_names),
                    out_names=tuple(out_names),
                    lowering_input_output_aliases=(),
                    sim_require_finite=True,
                    sim_require_nnan=True,
                    nc=nc,
                )
            )

        devices = jax.devices()[:n_cores]
        assert len(devices) >= n_cores, f"need {n_cores} devices, got {len(devices)}"
        mesh = Mesh(np.asarray(devices), ("core",))
        self.sharding = NamedSharding(mesh, PartitionSpec("core"))
        self.sharded = jax.jit(
            shard_map(
                _body, mesh=mesh,
                in_specs=(PartitionSpec("core"),) * (n_params + len(out_names)),
                out_specs=(PartitionSpec("core"),) * len(out_names),
                check_rep=False,
            ),
            donate_argnums=donate, keep_unused=True,
        )
        self._outs = None

    def __call__(self, in_maps):
        jax = self.jax
        per_core = [[np.asarray(m[name]) for name in self.in_names] for m in in_maps]
        concat_in = [
            np.concatenate([per_core[c][i] for c in range(self.n_cores)], axis=0)
            for i in range(len(self.in_names))
        ]
        xin = [jax.device_put(a, self.sharding) for a in concat_in]
        if self._outs is None:
            self._outs = [
                jax.device_put(
                    np.zeros((self.n_cores * av.shape[0], *av.shape[1:]), av.dtype),
                    self.sharding,
                )
                for av in self.out_avals
            ]
        self._outs = list(self.sharded(*xin, *self._outs))
        out_np = [np.asarray(o) for o in self._outs]
        return [
            {
                name: out_np[i].reshape(self.n_cores, *self.out_avals[i].shape)[c]
                for i, name in enumerate(self.out_names)
            }
            for c in range(self.n_cores)
        ]


def _get_nc(mode: str = "dr"):
    key = (mode, LOOP_MODE)
    if key not in _nc_cache:
        _nc_cache[key] = _build(mode)
    return _nc_cache[key]


def _run_spmd(nc, in_maps, mode: str = "dr"):
    last = None
    key = (mode, LOOP_MODE)
    for attempt in range(3):
        try:
            if key not in _runner_cache:
                _runner_cache[key] = _Runner(nc, NCORES)
            return _runner_cache[key](in_maps)
        except Exception as e:
            last = e
            _runner_cache.pop(key, None)
        try:
            from concourse.bass_utils import run_bass_kernel_spmd

            return run_bass_kernel_spmd(nc, in_maps, list(range(NCORES))).results
        except Exception as e:
            last = e
            _time.sleep(15)
    raise last


def kernel(x, reweight, conv_w):
    nc = _get_nc()
    in_maps = make_in_maps(x, reweight, conv_w)
    results = _run_spmd(nc, in_maps)
    out = np.concatenate([results[i]["out"] for i in range(NCORES)], axis=0)
    return np.ascontiguousarray(out.astype(np.float32))


# revision 6
# speedup vs baseline: 1.2290x; 1.0898x over previous
"""Binarized 3x3 conv via mixed-precision taps on 8 TRN2 cores.

Math: out = conv2d(clip(x,-1,1), concat(clip(reweight,-1,1), conv_w)), pad=1
= one 128->128 3x3 conv.  Data-parallel: 4 images/core, weights replicated.

Per-core kernel: 4 of the 9 taps are computed with e4m3 fp8 operands in
DoubleRowSwInterleave perf mode (2 taps per matmul with host-pre-interleaved
weights for contiguous weight loads); the remaining 5 taps run in fp16
(exact).  The fp8 taps are the four corners, paired vertically so both
pairs share one plane-shift delta.  Verified absmax-rel error vs the f32
reference: 1.864e-2 (< 2e-2), bit-identical to the host-side numpy
simulation of the same quantization.

Host pre-clips and pre-quantizes activations; fp8 activations are DMA'd into
2 spatially-shifted copies ("planes") of a padded SBUF buffer so a DoubleRow
pair (2 taps at different offsets) reads the two planes with one natural
strided AP [C, 2, R, W].  Each input tensor moves as ONE whole-image DMA
(fewest descriptors/semaphores, maximal slack against the DMA->PE read-
visibility race).  Input ships as fp8+fp16 (~25 MB/core vs 25.7 f32) and
output as fp16, cutting total HBM traffic ~25%% vs the f32 baseline.

Matmuls are issued tap-outer over groups of 8 PSUM banks ("group" loop) so
each weight load serves 8 matmuls — weight loads are NOT hidden behind
matmuls on this setup (measured +53ns/matmul fp16, +213ns per DoubleRow
load), so amortizing and shrinking them is what the loop structure and the
SwInterleave layout buy.  Measured (chained-dispatch slope): 181 us/rep
(204-216 with 3 planes + 16-row-chunked DMA; 232 for the naive block loop;
~250-265 for the staged fp16 baseline on the same hardware session).

Per-channel scales: weights are scaled by s_co/s_ci before quantization and
activations by s_ci; the PSUM->SBUF copy multiplies by 1/s_co per partition.
fp16-tap weights carry s_co so all taps accumulate at the same scale.
"""

import time as _time

import numpy as np
import ml_dtypes
from contextlib import ExitStack

import concourse.bass as bass
import concourse.mybir as mybir
import concourse.tile as tile
from concourse import bacc

B, C, H, W = 32, 128, 112, 112
NCORES = 8
BPC = B // NCORES
R = 4  # output rows per PSUM block

GUARD = 2          # extra zero margin (rows/cols) so shifted planes stay in-bounds
HP = H + 2 + 2 * GUARD   # 118 buffer rows
WP = 128                 # padded row stride (bytes for fp8); cols 0..117 used
ROW0 = GUARD + 1         # buffer row of image row 0 (plane shift 0)
COL0 = GUARD + 1         # buffer col of image col 0

# ---- tap config (overridden by the optimizer via set_config) ----
# pairs: list of ((khA,kwA),(khB,kwB), plane_lo); requires
#   SIGMA[plane_lo+1]-SIGMA[plane_lo] == tapA - tapB
SIGMA = [(0, 0), (-2, 0)]
PAIRS = [((-1, -1), (1, -1), 0), ((-1, 1), (1, 1), 0)]
TAPS16 = [(-1, 0), (0, -1), (0, 0), (0, 1), (1, 0)]
S_CO = np.ones(C, np.float32)
S_CI = np.ones(C, np.float32)
OUT_DT = "fp16"  # "fp16" | "f32"

LOOP_MODE = "group"  # "block" | "group" (tap-outer over 8 PSUM banks)
GROUP = 8
PAIR_PERF = "drsw"  # "dr" | "drsw" (DoubleRowSwInterleave: contiguous weight loads)

MODE = "dr"

_nc_cache: dict = {}
_runner_cache: dict = {}


def set_config(sigma, pairs, taps16, s_co, s_ci, out_dt="fp16"):
    global SIGMA, PAIRS, TAPS16, S_CO, S_CI, OUT_DT, _nc_cache, _runner_cache
    SIGMA, PAIRS, TAPS16 = sigma, pairs, taps16
    S_CO = np.asarray(s_co, np.float32)
    S_CI = np.asarray(s_ci, np.float32)
    OUT_DT = out_dt
    _nc_cache, _runner_cache = {}, {}


def _build(mode: str = "dr", bpc: int = BPC, h: int = H, w: int = W, reps: int = 1) -> bass.Bass:
    f32 = mybir.dt.float32
    fp16 = mybir.dt.float16
    fp8 = mybir.dt.float8e4
    out_dt = fp16 if OUT_DT == "fp16" else f32
    NP = len(SIGMA)
    NPAIR = len(PAIRS)
    N16 = len(TAPS16)
    assert h % R == 0

    w8_shape = (
        [C, max(NPAIR, 1), C, 2] if PAIR_PERF == "drsw" else [C, max(NPAIR, 1), 2, C]
    )
    nc = bacc.Bacc("TRN2", target_bir_lowering=False, debug=False)
    x8_in = nc.declare_dram_parameter("x8", [bpc, C, h, w], fp8, isOutput=False)
    x16_in = nc.declare_dram_parameter("x16", [bpc, C, h, w], fp16, isOutput=False)
    w8_in = nc.declare_dram_parameter("w8", w8_shape, fp8, isOutput=False)
    w16_in = nc.declare_dram_parameter("w16", [C, max(N16, 1), C], fp16, isOutput=False)
    sinv_in = nc.declare_dram_parameter("sinv", [C, 1], f32, isOutput=False)
    out_d = nc.declare_dram_parameter("out", [bpc, C, h, w], out_dt, isOutput=True)

    pair_perf = (
        mybir.MatmulPerfMode.DoubleRowSwInterleave
        if PAIR_PERF == "drsw"
        else mybir.MatmulPerfMode.DoubleRow
    )
    with tile.TileContext(nc) as tc, ExitStack() as ctx:
        wpool = ctx.enter_context(tc.tile_pool(name="wpool", bufs=1))
        apool = ctx.enter_context(tc.tile_pool(name="apool", bufs=1))
        opool = ctx.enter_context(tc.tile_pool(name="opool", bufs=3))
        pspool = ctx.enter_context(tc.tile_pool(name="pspool", bufs=8, space="PSUM"))

        w8_s = wpool.tile(w8_shape, fp8)
        w16_s = wpool.tile([C, max(N16, 1), C], fp16)
        sinv_s = wpool.tile([C, 1], f32)
        nc.sync.dma_start(w8_s[:], w8_in[:])
        nc.sync.dma_start(w16_s[:], w16_in[:])
        nc.sync.dma_start(sinv_s[:], sinv_in[:])

        # PE warmup against the HAM clock gate during the input-DMA lead-in
        warm = wpool.tile([C, R * w], fp16)
        nc.vector.memset(warm[:], 0.0)
        wps = pspool.tile([C, R * w], f32, tag="ps")
        for _wi in range(6):
            nc.tensor.matmul(wps[:], warm[:, :C], warm[:], start=True, stop=True)

        for b_outer in range(bpc * reps):
            b = b_outer % bpc
            a8 = apool.tile([C, NP, HP, WP], fp8, tag="a8", bufs=2)
            a16 = apool.tile([C, HP, WP], fp16, tag="a16", bufs=2)

            # zero the guard borders of each plane (only rows/cols a tap can read)
            for p in range(NP):
                sh, sw = SIGMA[p]
                r_img0, r_img1 = ROW0 + sh, ROW0 + sh + h  # image row span in buffer
                c_img0, c_img1 = COL0 + sw, COL0 + sw + w
                nc.vector.memset(a8[:, p, 0:r_img0, :118], 0.0)
                nc.vector.memset(a8[:, p, r_img1:118, :118], 0.0)
                nc.vector.memset(a8[:, p, r_img0:r_img1, 0:c_img0], 0.0)
                nc.vector.memset(a8[:, p, r_img0:r_img1, c_img1:118], 0.0)
            nc.vector.memset(a16[:, 0:ROW0, :118], 0.0)
            nc.vector.memset(a16[:, ROW0 + h : 118, :118], 0.0)
            nc.vector.memset(a16[:, ROW0 : ROW0 + h, 0:COL0], 0.0)
            nc.vector.memset(a16[:, ROW0 : ROW0 + h, COL0 + w : 118], 0.0)

            sched = [h]  # one chunk per tensor: fewest DMA descriptors/sems,
            # and maximal slack between a chunk's semaphore and first PE read
            r0 = 0
            for sz in sched:
                r1 = r0 + sz
                for p in range(NP):
                    sh, sw = SIGMA[p]
                    nc.sync.dma_start(
                        a8[:, p, ROW0 + sh + r0 : ROW0 + sh + r1, COL0 + sw : COL0 + sw + w],
                        x8_in[b][:, r0:r1, :],
                    )
                nc.sync.dma_start(
                    a16[:, ROW0 + r0 : ROW0 + r1, COL0 : COL0 + w],
                    x16_in[b][:, r0:r1, :],
                )
                r0 = r1

            out_flat = out_d[b].rearrange("c h w -> c (h w)")
            n_mm = NPAIR + N16

            def dr_rhs(j, h0):
                tA, tB, pl = PAIRS[j]
                khA, kwA = tA
                sh, sw = SIGMA[pl]
                rb = ROW0 + h0 + khA + sh
                cb = COL0 + kwA + sw
                return a8[:, pl : pl + 2, rb : rb + R, cb : cb + w]

            def f16_rhs(i, h0):
                kh, kw = TAPS16[i]
                return a16[:, ROW0 + h0 + kh : ROW0 + h0 + kh + R,
                           COL0 + kw : COL0 + kw + w]

            def emit_copy(ps, h0):
                ot = opool.tile([C, R * w], out_dt)
                nc.scalar.activation(
                    ot[:], ps[:], mybir.ActivationFunctionType.Copy,
                    scale=sinv_s[:],
                )
                nc.scalar.dma_start(out_flat[:, h0 * w : (h0 + R) * w], ot[:])

            if LOOP_MODE == "block":
                for h0 in range(0, h, R):
                    ps = pspool.tile([C, R * w], f32)
                    i_mm = 0
                    for j in range(NPAIR):
                        nc.tensor.matmul(
                            ps[:], w8_s[:, j], dr_rhs(j, h0),
                            start=(i_mm == 0), stop=(i_mm == n_mm - 1),
                            perf_mode=pair_perf,
                        )
                        i_mm += 1
                    for i in range(N16):
                        nc.tensor.matmul(
                            ps[:], w16_s[:, i], f16_rhs(i, h0),
                            start=(i_mm == 0), stop=(i_mm == n_mm - 1),
                        )
                        i_mm += 1
                    emit_copy(ps, h0)
            else:  # "group": tap-outer over GROUP psum banks, 1 weight load per tap
                for g0 in range(0, h, R * GROUP):
                    h0s = list(range(g0, min(g0 + R * GROUP, h), R))
                    pss = [
                        pspool.tile([C, R * w], f32, name="ps", tag="ps")
                        for _ in h0s
                    ]
                    i_mm = 0
                    for j in range(NPAIR):
                        for bi, h0 in enumerate(h0s):
                            nc.tensor.matmul(
                                pss[bi][:], w8_s[:, j], dr_rhs(j, h0),
                                start=(i_mm == 0), stop=(i_mm == n_mm - 1),
                                perf_mode=pair_perf,
                                skip_group_check=True,
                            )
                        i_mm += 1
                    for i in range(N16):
                        for bi, h0 in enumerate(h0s):
                            nc.tensor.matmul(
                                pss[bi][:], w16_s[:, i], f16_rhs(i, h0),
                                start=(i_mm == 0), stop=(i_mm == n_mm - 1),
                                skip_group_check=True,
                            )
                        i_mm += 1
                    otg = opool.tile(
                        [C, len(h0s) * R * w], out_dt, name="otg", tag="otg"
                    )
                    for bi, h0 in enumerate(h0s):
                        nc.scalar.activation(
                            otg[:, bi * R * w : (bi + 1) * R * w], pss[bi][:],
                            mybir.ActivationFunctionType.Copy,
                            scale=sinv_s[:],
                        )
                    nc.scalar.dma_start(
                        out_flat[:, h0s[0] * w : (h0s[0] + len(h0s) * R) * w],
                        otg[:],
                    )

    nc.compile()
    return nc


def _prep_weights(reweight: np.ndarray, conv_w: np.ndarray):
    """Returns (w8, w16, sinv) device tensors from the full-precision weights."""
    w_full = np.concatenate(
        [np.clip(reweight, -1.0, 1.0), conv_w], axis=0
    ).astype(np.float32)  # [co, ci, 3, 3]
    NPAIR, N16 = len(PAIRS), len(TAPS16)
    # effective per-weight scale: s_co / s_ci  (activation carries s_ci)
    g = S_CO[None, :] / S_CI[:, None]  # [ci, co]
    if PAIR_PERF == "drsw":
        # interleaved layout: [ci, pair, p, j]: elem (p, 0) = A[:, 127-p],
        # (p, 1) = B[:, 127-p]  (A/B pairs adjacent, columns reversed)
        w8 = np.zeros((C, max(NPAIR, 1), C, 2), ml_dtypes.float8_e4m3)
        for j, (tA, tB, _pl) in enumerate(PAIRS):
            for k, (kh, kw) in enumerate((tA, tB)):
                q = (w_full[:, :, kh + 1, kw + 1].T * g).astype(
                    ml_dtypes.float8_e4m3
                )  # [ci, co]
                w8[:, j, :, k] = q[:, ::-1]
    else:
        w8 = np.zeros((C, max(NPAIR, 1), 2, C), ml_dtypes.float8_e4m3)
        for j, (tA, tB, _pl) in enumerate(PAIRS):
            for k, (kh, kw) in enumerate((tA, tB)):
                # lhsT layout [ci, pair, k, co]
                w8[:, j, k, :] = (w_full[:, :, kh + 1, kw + 1].T * g).astype(
                    ml_dtypes.float8_e4m3
                )
    w16 = np.zeros((C, max(N16, 1), C), np.float16)
    for i, (kh, kw) in enumerate(TAPS16):
        w16[:, i, :] = (w_full[:, :, kh + 1, kw + 1].T * S_CO[None, :]).astype(
            np.float16
        )
    sinv = (1.0 / S_CO).astype(np.float32).reshape(C, 1)
    return w8, w16, sinv


def _prep_acts(x: np.ndarray):
    a = np.clip(np.asarray(x, np.float32), -1.0, 1.0)
    a8 = (a * S_CI[None, :, None, None]).astype(ml_dtypes.float8_e4m3)
    a16 = a.astype(np.float16)
    return a8, a16


def make_in_maps(x, reweight, conv_w):
    w8, w16, sinv = _prep_weights(np.asarray(reweight), np.asarray(conv_w))
    a8, a16 = _prep_acts(x)
    return [
        {
            "x8": np.ascontiguousarray(a8[i * BPC : (i + 1) * BPC]),
            "x16": np.ascontiguousarray(a16[i * BPC : (i + 1) * BPC]),
            "w8": w8, "w16": w16, "sinv": sinv,
        }
        for i in range(NCORES)
    ]


class _Runner:
    """Persistent jitted shard_map executor (same as baseline kernel.py)."""

    def __init__(self, nc, n_cores: int):
        import jax
        from concourse import bass2jax
        from jax.experimental.shard_map import shard_map
        from jax.sharding import Mesh, NamedSharding, PartitionSpec

        bass2jax.install_neuronx_cc_hook()
        self.jax = jax
        self.n_cores = n_cores
        partition_name = nc.partition_id_tensor.name if nc.partition_id_tensor else None
        in_names, out_names, out_avals = [], [], []
        for alloc in nc.m.functions[0].allocations:
            if not isinstance(alloc, mybir.MemoryLocationSet):
                continue
            name = alloc.memorylocations[0].name
            if alloc.kind == "ExternalInput":
                if name != partition_name:
                    in_names.append(name)
            elif alloc.kind == "ExternalOutput":
                out_names.append(name)
                out_avals.append(
                    jax.core.ShapedArray(
                        tuple(alloc.tensor_shape), mybir.dt.np(alloc.dtype)
                    )
                )
        self.in_names, self.out_names, self.out_avals = in_names, out_names, out_avals
        n_params = len(in_names)
        all_in_names = list(in_names) + list(out_names)
        if partition_name is not None:
            all_in_names.append(partition_name)
        donate = tuple(range(n_params, n_params + len(out_names)))

        def _body(*args):
            operands = list(args)
            if partition_name is not None:
                operands.append(bass2jax.partition_id_tensor())
            return tuple(
                bass2jax._bass_exec_p.bind(
                    *operands,
                    out_avals=tuple(out_avals),
                    in_names=tuple(all_in_names),
                    out_names=tuple(out_names),
                    lowering_input_output_aliases=(),
                    sim_require_finite=True,
                    sim_require_nnan=True,
                    nc=nc,
                )
            )

        devices = jax.devices()[:n_cores]
        assert len(devices) >= n_cores, f"need {n_cores} devices, got {len(devices)}"
        mesh = Mesh(np.asarray(devices), ("core",))
        self.sharding = NamedSharding(mesh, PartitionSpec("core"))
        self.sharded = jax.jit(
            shard_map(
                _body, mesh=mesh,
                in_specs=(PartitionSpec("core"),) * (n_params + len(out_names)),
                out_specs=(PartitionSpec("core"),) * len(out_names),
                check_rep=False,
            ),
            donate_argnums=donate, keep_unused=True,
        )
        self._outs = None

    def __call__(self, in_maps):
        jax = self.jax
        per_core = [[np.asarray(m[name]) for name in self.in_names] for m in in_maps]
        concat_in = [
            np.concatenate([per_core[c][i] for c in range(self.n_cores)], axis=0)
            for i in range(len(self.in_names))
        ]
        xin = [jax.device_put(a, self.sharding) for a in concat_in]
        if self._outs is None:
            self._outs = [
                jax.device_put(
                    np.zeros((self.n_cores * av.shape[0], *av.shape[1:]), av.dtype),
                    self.sharding,
                )
                for av in self.out_avals
            ]
        self._outs = list(self.sharded(*xin, *self._outs))
        out_np = [np.asarray(o) for o in self._outs]
        return [
            {
                name: out_np[i].reshape(self.n_cores, *self.out_avals[i].shape)[c]
                for i, name in enumerate(self.out_names)
            }
            for c in range(self.n_cores)
        ]


def _get_nc(mode: str = "dr"):
    key = (mode, LOOP_MODE)
    if key not in _nc_cache:
        _nc_cache[key] = _build(mode)
    return _nc_cache[key]


def _run_spmd(nc, in_maps, mode: str = "dr"):
    last = None
    key = (mode, LOOP_MODE)
    for attempt in range(3):
        try:
            if key not in _runner_cache:
                _runner_cache[key] = _Runner(nc, NCORES)
            return _runner_cache[key](in_maps)
        except Exception as e:
            last = e
            _runner_cache.pop(key, None)
        try:
            from concourse.bass_utils import run_bass_kernel_spmd

            return run_bass_kernel_spmd(nc, in_maps, list(range(NCORES))).results
        except Exception as e:
            last = e
            _time.sleep(15)
    raise last


def kernel(x, reweight, conv_w):
    nc = _get_nc()
    in_maps = make_in_maps(x, reweight, conv_w)
    results = _run_spmd(nc, in_maps)
    out = np.concatenate([results[i]["out"] for i in range(NCORES)], axis=0)
    return np.ascontiguousarray(out.astype(np.float32))
